# revision 10
# baseline (speedup 1.0000x reference)
"""AnchorBasedTrajectoryDecoder on 8 TRN2 NeuronCores (Bass/Tile).

Data-parallel: batch axis (1024) split into 8 shards of 128 rows; each core
runs all K=6 anchors for its rows => N = 768 independent LSTM rollouts per
core, laid out feature-major (columns j = k*128 + bs, "k-major").

Device program per core:
  encoder:  combined^T -> hidden^T (LeakyRelu) -> endpoint/conf, gx0
  loop t:   layer0 gates = gx0 + W_hh0 @ h0   (gx0 injected into PSUM by an
            identity matmul, recurrent matmuls accumulate on top)
            layer1 gates = [W_ih1|W_hh1] @ [h_l0(t); h_l1(t-1)]  (K=512)
            cell updates on ACT (sigmoid/tanh) + DVE (elementwise)
            output projection time-blocked 8 steps per PSUM accumulation
            via a block-diagonal W_op stack (M=16)
  layer1 is software-staggered one step behind layer0 so the ACT/DVE tail of
  each layer hides under the other layer's matmul block.
"""

import numpy as np
import ml_dtypes
from contextlib import ExitStack

import concourse.bacc as bacc
import concourse.bass as bass
import concourse.mybir as mybir
import concourse.tile as tile
from concourse.bass_utils import run_bass_kernel_spmd
from concourse.masks import make_identity

# problem constants (hardcoded from the spec)
BS, OBS, LANE, DSH = 1024, 128, 64, 64
IN_DIM, H, K, T = 256, 256, 6, 30
H4 = 4 * H
NCORES = 8
BS_C = BS // NCORES          # 128 batch rows per core
N = BS_C * K                 # 768 rollouts per core
NCHUNKS = ((0, 512), (512, 256))   # PSUM-bank-aligned column chunks of N
GS = 8                       # time-group size for the output projection
NGROUPS = (T + GS - 1) // GS

F32 = mybir.dt.float32
F32R = mybir.dt.float32r
BF16 = mybir.dt.bfloat16
AF = mybir.ActivationFunctionType
SIG, TANH, IDENT, LRELU = AF.Sigmoid, AF.Tanh, AF.Identity, AF.Lrelu

# --- config flags -----------------------------------------------------------
import os
# "f32": f32 storage + f32r matmuls, f32 cell (max precision)
# "bf16": bf16 matmuls (FWL weight loads) + bf16 cell (2x DVE modes)
PRECISION = os.environ.get("KPREC", "bf16")
# how many of the 8 layer-0 m-tiles inject gx0 via the PE identity-matmul
# (the rest use a DVE add from PSUM); bf16 mode defaults to all-DVE since
# the bf16 cell frees DVE capacity and the PE queue is the bottleneck
GX0_PE_PAIRS = int(os.environ.get("KGX0PE", "4" if PRECISION == "f32" else "0"))


def _np_stor():
    return np.float32 if PRECISION == "f32" else ml_dtypes.bfloat16


def _prep_host(inp):
    """Host-side weight re-layout (f64 math, cast to storage dtype)."""
    g = lambda k: np.asarray(inp[k], np.float64)
    W_hid, b_hid = g("W_hid"), g("b_hid")
    W_ep, b_ep = g("W_ep"), g("b_ep")
    W_conf, b_conf = g("W_conf"), g("b_conf")
    W_ih0, b_ih0 = g("W_ih0"), g("b_ih0")
    W_hh0, b_hh0 = g("W_hh0"), g("b_hh0")
    W_ih1, b_ih1 = g("W_ih1"), g("b_ih1")
    W_hh1, b_hh1 = g("W_hh1"), g("b_hh1")
    W_op, b_op = g("W_op"), g("b_op")

    # endpoint is a linear function of hidden; fold it into the layer-0 input
    # projection:  gx0 = W_eff @ hidden + b_eff
    W_eff = W_ih0[:, :H] + W_ih0[:, H:H + 2] @ W_ep          # [4H, H]
    b_eff = b_ih0 + b_hh0 + W_ih0[:, H:H + 2] @ b_ep         # [4H]

    def ktiles(WT):  # [K, M] -> [128, K/128, M]
        Kd, M = WT.shape
        assert Kd % 128 == 0
        return np.ascontiguousarray(WT.reshape(Kd // 128, 128, M).transpose(1, 0, 2))

    W_opT = W_op.T                                            # [H, 2]
    a_op = np.zeros((16, 128, 16))
    for tc in range(GS):
        for kk in range(2):
            a_op[tc * 2 + kk, :, 2 * tc:2 * tc + 2] = W_opT[kk * 128:(kk + 1) * 128]
    a_op = np.ascontiguousarray(a_op.transpose(1, 0, 2))      # [128, 16, 16]

    stor = _np_stor()
    A = {
        "a_hid": ktiles(W_hid.T).astype(stor),                # [128, 2, 256]
        "a_gx0": ktiles(W_eff.T).astype(stor),                # [128, 2, 1024]
        "a_hh0": ktiles(W_hh0.T).astype(stor),                # [128, 2, 1024]
        "a_l1": ktiles(np.concatenate([W_ih1, W_hh1], 1).T).astype(stor),  # [128,4,1024]
        "a_ec": ktiles(np.concatenate([W_ep, W_conf], 0).T).astype(stor),  # [128,2,3]
        "a_op": a_op.astype(stor),                            # [128, 16, 16]
        "b_eff": np.ascontiguousarray(b_eff.reshape(8, 128).T).astype(np.float32),
        "b_g1": np.ascontiguousarray((b_ih1 + b_hh1).reshape(8, 128).T).astype(np.float32),
        "b_hid": np.ascontiguousarray(b_hid.reshape(2, 128).T).astype(np.float32),
        "b_ec": np.concatenate([b_ep, b_conf]).reshape(3, 1).astype(np.float32),
        "b_op": np.tile(b_op, GS).reshape(16, 1).astype(np.float32),
    }
    return A


def _build():
    """Build the per-core Bass program (identical on all cores)."""
    # dtype for matmul-feeding tensors: FP32r tensors must be *written* as
    # f32r (the verifier enforces producer-side rounding), so declare the
    # tiles and dram params with that dtype directly.
    stor = F32R if PRECISION == "f32" else BF16
    cdt = F32 if PRECISION == "f32" else BF16   # cell/activation tile dtype
    # gx0 feeds the PE identity-matmul (needs f32r) or only DVE adds (bf16 ok)
    gxdt = F32R if GX0_PE_PAIRS > 0 else (F32 if PRECISION == "f32" else BF16)

    nc = bacc.Bacc("TRN2", target_bir_lowering=False, debug=False)

    d_obs = nc.dram_tensor("obs", [BS_C, OBS], F32, kind="ExternalInput")
    d_lane = nc.dram_tensor("lane", [BS_C, LANE], F32, kind="ExternalInput")
    d_ds = nc.dram_tensor("ds", [K, DSH], F32, kind="ExternalInput")
    d_w = {}
    wshapes = {
        "a_hid": [128, 2, H], "a_gx0": [128, 2, H4], "a_hh0": [128, 2, H4],
        "a_l1": [128, 4, H4], "a_ec": [128, 2, 3], "a_op": [128, 16, 16],
    }
    for k, sh in wshapes.items():
        d_w[k] = nc.dram_tensor(k, sh, stor, kind="ExternalInput")
    bshapes = {"b_eff": [128, 8], "b_g1": [128, 8], "b_hid": [128, 2],
               "b_ec": [3, 1], "b_op": [16, 1]}
    for k, sh in bshapes.items():
        d_w[k] = nc.dram_tensor(k, sh, F32, kind="ExternalInput")

    d_traj = nc.dram_tensor("out_traj", [NGROUPS, 16, N], F32, kind="ExternalOutput")
    d_ec = nc.dram_tensor("out_ec", [3, N], F32, kind="ExternalOutput")

    with ExitStack() as ctx:
        tc_ = ctx.enter_context(tile.TileContext(nc))
        wpool = ctx.enter_context(tc_.tile_pool(name="weights", bufs=1))
        enc = ctx.enter_context(tc_.tile_pool(name="enc", bufs=1))
        gx0p = ctx.enter_context(tc_.tile_pool(name="gx0", bufs=1))
        hp0 = ctx.enter_context(tc_.tile_pool(name="h0", bufs=3))
        hp1 = ctx.enter_context(tc_.tile_pool(name="h1", bufs=3))
        cp = ctx.enter_context(tc_.tile_pool(name="cstate", bufs=1))
        actp = ctx.enter_context(tc_.tile_pool(name="acts", bufs=12))
        prep = ctx.enter_context(tc_.tile_pool(name="preact", bufs=3))
        tmpp = ctx.enter_context(tc_.tile_pool(name="tmp", bufs=3))
        tcp = ctx.enter_context(tc_.tile_pool(name="tanhc", bufs=3))
        stag = ctx.enter_context(tc_.tile_pool(name="stage", bufs=2))
        # PSUM: gate pool 3 x [128,768] tiles (2 banks each) + proj (2 banks)
        gp = ctx.enter_context(tc_.tile_pool(name="gpsum", bufs=3, space="PSUM"))
        pp = ctx.enter_context(tc_.tile_pool(name="ppsum", bufs=1, space="PSUM"))

        # ---- inputs first (tiny), then weights ordered by first use --------
        obs_sb = enc.tile([128, OBS], F32, tag="obs")
        nc.sync.dma_start(out=obs_sb, in_=d_obs[:])
        lane_sb = enc.tile([128, LANE], F32, tag="lane")
        nc.sync.dma_start(out=lane_sb, in_=d_lane[:])
        dsT = enc.tile([128, K], F32, tag="dsT")          # rows 64..127 used
        nc.sync.dma_start(out=dsT[64:128, :], in_=d_ds[:].rearrange("k d -> d k"))

        ident = wpool.tile([128, 128], F32, tag="ident")
        make_identity(nc, ident)
        identr = wpool.tile([128, 128], F32R, tag="identr")
        nc.vector.tensor_copy(identr, ident)

        w = {}
        for k, sh in bshapes.items():
            w[k] = wpool.tile(sh, F32, tag=k, name=k)
            nc.sync.dma_start(out=w[k], in_=d_w[k][:])
        order = ["a_hid", "a_ec", "a_gx0", "a_hh0", "a_l1", "a_op"]
        for k in order:
            w[k] = wpool.tile(wshapes[k], stor, tag=k, name=k)
            nc.sync.dma_start(out=w[k], in_=d_w[k][:])

        # ---- encoder -------------------------------------------------------

        # transposes via PE
        obsT_ps = gp.tile([128, 128], F32, tag="g")
        nc.tensor.transpose(obsT_ps, obs_sb, ident)
        obsT = enc.tile([128, 128], stor, tag="obsT")
        nc.vector.tensor_copy(obsT, obsT_ps)
        laneT_ps = gp.tile([64, 128], F32, tag="g")
        nc.tensor.transpose(laneT_ps, lane_sb, ident)
        laneT = enc.tile([64, 128], stor, tag="laneT")
        nc.vector.tensor_copy(laneT, laneT_ps)

        # combined^T [256, 768]: rows 0:128 obs, 128:192 lane, 192:256 ds
        combT = enc.tile([128, 2, N], stor, tag="combT")
        for k in range(K):
            blk = slice(k * 128, (k + 1) * 128)
            nc.vector.tensor_copy(combT[:, 0, blk], obsT)
            nc.vector.tensor_copy(combT[0:64, 1, blk], laneT)
            # broadcast ds[k] along the 128 batch columns (in0 * 0 + ds)
            nc.vector.tensor_scalar(
                out=combT[64:128, 1, blk], in0=obsT[64:128, 0:128],
                scalar1=0.0, scalar2=dsT[64:128, k:k + 1],
                op0=mybir.AluOpType.mult, op1=mybir.AluOpType.add)

        # hidden^T = LeakyRelu(W_hid @ combined^T + b_hid)   [256, 768]
        hidT = enc.tile([128, 2, N], stor, tag="hidT")
        for m in range(2):
            ps = gp.tile([128, N], F32, tag="g")
            for (n0, nw) in NCHUNKS:
                for kk in range(2):
                    nc.tensor.matmul(
                        ps[:, n0:n0 + nw], w["a_hid"][:, kk, m * 128:(m + 1) * 128],
                        combT[:, kk, n0:n0 + nw], start=(kk == 0), stop=(kk == 1))
            # leaky_relu(r, 0.1) = max(r, 0.1*r); Lrelu is unimplemented in sim
            r_ = enc.tile([128, N], F32, tag="enc_r", name="enc_r")
            nc.scalar.activation(r_, ps, IDENT, bias=w["b_hid"][:, m:m + 1])
            s_ = enc.tile([128, N], F32, tag="enc_s", name="enc_s")
            nc.vector.tensor_scalar_mul(s_, r_, 0.1)
            nc.vector.tensor_max(hidT[:, m, :], r_, s_)

        # endpoint / conf : [3, 768]
        ecps = gp.tile([3, N], F32, tag="g")
        for (n0, nw) in NCHUNKS:
            for kk in range(2):
                nc.tensor.matmul(ecps[:, n0:n0 + nw], w["a_ec"][:, kk, :],
                                 hidT[:, kk, n0:n0 + nw],
                                 start=(kk == 0), stop=(kk == 1))
        ec_st = stag.tile([3, N], F32, tag="ec")
        nc.scalar.activation(ec_st, ecps, IDENT, bias=w["b_ec"][:, 0:1])
        nc.sync.dma_start(out=d_ec[:], in_=ec_st)

        # gx0 = W_eff @ hidden^T + b_eff   [1024, 768] f32, kept in SBUF
        gx0 = gx0p.tile([128, 8, N], gxdt, tag="gx0")
        for m in range(8):
            ps = gp.tile([128, N], F32, tag="g")
            for (n0, nw) in NCHUNKS:
                for kk in range(2):
                    nc.tensor.matmul(ps[:, n0:n0 + nw],
                                     w["a_gx0"][:, kk, m * 128:(m + 1) * 128],
                                     hidT[:, kk, n0:n0 + nw],
                                     start=(kk == 0), stop=(kk == 1))
            nc.scalar.activation(gx0[:, m, :], ps, IDENT, bias=w["b_eff"][:, m:m + 1])

        # ---- LSTM loop -----------------------------------------------------
        cA = cp.tile([128, 2, N], cdt, tag="cA")   # layer0 cell state
        cB = cp.tile([128, 2, N], cdt, tag="cB")   # layer1 cell state
        proj_ps = [None]

        def cell(acts, c, h_pool, h_tag, t):
            """acts[m] m=0..7 (i,i,f,f,g,g,o,o); returns new h tile."""
            h_new = h_pool.tile([128, 2, N], stor, tag=h_tag)
            for kk in range(2):
                s_i, s_f, g_g = acts[0 + kk], acts[2 + kk], acts[4 + kk]
                if t == 0:
                    nc.vector.tensor_mul(c[:, kk, :], s_i, g_g)
                else:
                    tmp = tmpp.tile([128, N], cdt, tag="tmp")
                    nc.vector.tensor_mul(tmp, s_i, g_g)
                    nc.vector.tensor_mul(c[:, kk, :], c[:, kk, :], s_f)
                    nc.vector.tensor_add(c[:, kk, :], c[:, kk, :], tmp)
            tch = tcp.tile([128, 2, N], cdt, tag="tc")
            nc.scalar.activation(tch, c, TANH)     # fused [128, 1536]
            for kk in range(2):
                nc.vector.tensor_mul(h_new[:, kk, :], acts[6 + kk], tch[:, kk, :])
            return h_new

        def l0_step(t, h_prev):
            acts = []
            for m in range(8):
                func = TANH if m // 2 == 2 else SIG
                a = actp.tile([128, N], cdt, tag="act")
                if t == 0:
                    nc.scalar.activation(a, gx0[:, m, :], func)
                else:
                    ps = gp.tile([128, N], F32, tag="g")
                    use_pe = m < GX0_PE_PAIRS * 2
                    for (n0, nw) in NCHUNKS:
                        if use_pe:  # inject gx0 via identity matmul
                            nc.tensor.matmul(ps[:, n0:n0 + nw], identr,
                                             gx0[:, m, n0:n0 + nw],
                                             start=True, stop=False,
                                             skip_group_check=True)
                        for kk in range(2):
                            nc.tensor.matmul(
                                ps[:, n0:n0 + nw],
                                w["a_hh0"][:, kk, m * 128:(m + 1) * 128],
                                h_prev[:, kk, n0:n0 + nw],
                                start=(not use_pe and kk == 0), stop=(kk == 1),
                                skip_group_check=True)
                    if use_pe:
                        nc.scalar.activation(a, ps, func)
                    else:
                        pre = prep.tile([128, N], cdt, tag="pre")
                        nc.vector.tensor_add(pre, ps, gx0[:, m, :])
                        nc.scalar.activation(a, pre, func)
                acts.append(a)
            return cell(acts, cA, hp0, "h0", t)

        def l1_step(s, h_in, h1s):
            acts = []
            for m in range(8):
                func = TANH if m // 2 == 2 else SIG
                ps = gp.tile([128, N], F32, tag="g")
                nk = 2 if s == 0 else 4
                for (n0, nw) in NCHUNKS:
                    for kt in range(nk):
                        rhs = h_in if kt < 2 else h1s
                        kk = kt % 2
                        nc.tensor.matmul(ps[:, n0:n0 + nw],
                                         w["a_l1"][:, kt, m * 128:(m + 1) * 128],
                                         rhs[:, kk, n0:n0 + nw],
                                         start=(kt == 0), stop=(kt == nk - 1),
                                         skip_group_check=True)
                a = actp.tile([128, N], cdt, tag="act")
                nc.scalar.activation(a, ps, func, bias=w["b_g1"][:, m:m + 1])
                acts.append(a)
            h1_new = cell(acts, cB, hp1, "h1", s)

            # ---- time-blocked output projection ----
            g_, tcg = divmod(s, GS)
            gsz = min(GS, T - g_ * GS)
            if tcg == 0:
                proj_ps[0] = pp.tile([16, N], F32, tag="proj", name="proj")
            pps = proj_ps[0]
            for (n0, nw) in NCHUNKS:
                for kk in range(2):
                    nc.tensor.matmul(pps[:, n0:n0 + nw], w["a_op"][:, tcg * 2 + kk, :],
                                     h1_new[:, kk, n0:n0 + nw],
                                     start=(tcg == 0 and kk == 0),
                                     stop=(tcg == gsz - 1 and kk == 1),
                                     skip_group_check=True)
            if tcg == gsz - 1:
                st = stag.tile([16, N], F32, tag="traj")
                nc.scalar.activation(st, pps, IDENT, bias=w["b_op"][:, 0:1])
                nc.sync.dma_start(out=d_traj[g_], in_=st)
            return h1_new

        h0_prev = None   # h_l0(t-1)
        h1_prev = None   # h_l1(s-1)
        for t in range(T):
            h0_new = l0_step(t, h0_prev)
            if t >= 1:
                h1_prev = l1_step(t - 1, h0_prev, h1_prev)
            h0_prev = h0_new
        h1_prev = l1_step(T - 1, h0_prev, h1_prev)

    nc.compile()
    return nc


def _shard_inputs(inp, weights):
    obs = np.ascontiguousarray(np.asarray(inp["obs_feature"], np.float32))
    lane = np.ascontiguousarray(np.asarray(inp["lane_change_feature"], np.float32))
    ds = np.ascontiguousarray(np.asarray(inp["driving_style"], np.float32).reshape(K, DSH))
    in_maps = []
    for c in range(NCORES):
        m = dict(weights)
        m["obs"] = obs[c * BS_C:(c + 1) * BS_C]
        m["lane"] = lane[c * BS_C:(c + 1) * BS_C]
        m["ds"] = ds
        in_maps.append(m)
    return in_maps


def _assemble(results):
    final = np.empty((BS, K, T, 2), np.float32)
    conf = np.empty((BS, K), np.float32)
    endpoint = np.empty((BS, K, 2), np.float32)
    for c, r in enumerate(results):
        traj = np.asarray(r["out_traj"])          # [4, 16, 768]
        # [g, tc, f, k, bs] -> [bs, k, g, tc, f]
        arr = traj.reshape(NGROUPS, GS, 2, K, BS_C).transpose(4, 3, 0, 1, 2)
        final[c * BS_C:(c + 1) * BS_C] = arr.reshape(BS_C, K, NGROUPS * GS, 2)[:, :, :T]
        ec = np.asarray(r["out_ec"]).reshape(3, K, BS_C)     # [3, k, bs]
        endpoint[c * BS_C:(c + 1) * BS_C] = ec[:2].transpose(2, 1, 0)
        conf[c * BS_C:(c + 1) * BS_C] = ec[2].T
    return final, conf, endpoint


_CACHE = {}


def run(inputs, trace=False):
    """Returns ((final, conf, endpoint), BassKernelResults)."""
    assert int(inputs.get("out_length", T)) == T
    if "nc" not in _CACHE:
        _CACHE["nc"] = _build()
    nc = _CACHE["nc"]
    weights = _prep_host(inputs)
    in_maps = _shard_inputs(inputs, weights)
    res = run_bass_kernel_spmd(nc, in_maps, core_ids=list(range(NCORES)),
                               trace=trace)
    return _assemble(res.results), res


def kernel(**inputs):
    out, _ = run(inputs)
    return out


# revision 11
# speedup vs baseline: 1.1979x; 1.1979x over previous
"""AnchorBasedTrajectoryDecoder on 8 TRN2 NeuronCores (Bass/Tile).

Data-parallel: batch axis (1024) split into 8 shards of 128 rows; each core
runs all K=6 anchors for its rows => N = 768 independent LSTM rollouts per
core, laid out feature-major (columns j = k*128 + bs, "k-major").

Device program per core:
  encoder:  combined^T -> hidden^T (LeakyRelu) -> endpoint/conf, gx0
  loop t:   layer0 gates = gx0 + W_hh0 @ h0   (gx0 injected into PSUM by an
            identity matmul, recurrent matmuls accumulate on top)
            layer1 gates = [W_ih1|W_hh1] @ [h_l0(t); h_l1(t-1)]  (K=512)
            cell updates on ACT (sigmoid/tanh) + DVE (elementwise)
            output projection time-blocked 8 steps per PSUM accumulation
            via a block-diagonal W_op stack (M=16)
  layer1 is software-staggered one step behind layer0 so the ACT/DVE tail of
  each layer hides under the other layer's matmul block.
"""

import numpy as np
import ml_dtypes
from contextlib import ExitStack

import concourse.bacc as bacc
import concourse.bass as bass
import concourse.mybir as mybir
import concourse.tile as tile
from concourse.bass_utils import run_bass_kernel_spmd
from concourse.masks import make_identity

# problem constants (hardcoded from the spec)
BS, OBS, LANE, DSH = 1024, 128, 64, 64
IN_DIM, H, K, T = 256, 256, 6, 30
H4 = 4 * H
NCORES = 8
BS_C = BS // NCORES          # 128 batch rows per core
N = BS_C * K                 # 768 rollouts per core
NCHUNKS = ((0, 512), (512, 256))   # PSUM-bank-aligned column chunks of N
GS = 8                       # time-group size for the output projection
NGROUPS = (T + GS - 1) // GS

F32 = mybir.dt.float32
F32R = mybir.dt.float32r
BF16 = mybir.dt.bfloat16
AF = mybir.ActivationFunctionType
SIG, TANH, IDENT, LRELU = AF.Sigmoid, AF.Tanh, AF.Identity, AF.Lrelu

# --- config flags -----------------------------------------------------------
import os
# "f32": f32 storage + f32r matmuls, f32 cell (max precision)
# "bf16": bf16 matmuls (FWL weight loads) + bf16 cell (2x DVE modes)
PRECISION = os.environ.get("KPREC", "bf16")
# how many of the 8 layer-0 m-tiles inject gx0 via the PE identity-matmul
# (the rest use a DVE add from PSUM); bf16 mode defaults to all-DVE since
# the bf16 cell frees DVE capacity and the PE queue is the bottleneck
GX0_PE_PAIRS = int(os.environ.get("KGX0PE", "4" if PRECISION == "f32" else "0"))


def _np_stor():
    return np.float32 if PRECISION == "f32" else ml_dtypes.bfloat16


def _prep_host(inp):
    """Host-side weight re-layout (f64 math, cast to storage dtype)."""
    g = lambda k: np.asarray(inp[k], np.float64)
    W_hid, b_hid = g("W_hid"), g("b_hid")
    W_ep, b_ep = g("W_ep"), g("b_ep")
    W_conf, b_conf = g("W_conf"), g("b_conf")
    W_ih0, b_ih0 = g("W_ih0"), g("b_ih0")
    W_hh0, b_hh0 = g("W_hh0"), g("b_hh0")
    W_ih1, b_ih1 = g("W_ih1"), g("b_ih1")
    W_hh1, b_hh1 = g("W_hh1"), g("b_hh1")
    W_op, b_op = g("W_op"), g("b_op")

    # endpoint is a linear function of hidden; fold it into the layer-0 input
    # projection:  gx0 = W_eff @ hidden + b_eff
    W_eff = W_ih0[:, :H] + W_ih0[:, H:H + 2] @ W_ep          # [4H, H]
    b_eff = b_ih0 + b_hh0 + W_ih0[:, H:H + 2] @ b_ep         # [4H]

    def ktiles(WT):  # [K, M] -> [128, K/128, M]
        Kd, M = WT.shape
        assert Kd % 128 == 0
        return np.ascontiguousarray(WT.reshape(Kd // 128, 128, M).transpose(1, 0, 2))

    W_opT = W_op.T                                            # [H, 2]
    a_op = np.zeros((16, 128, 16))
    for tc in range(GS):
        for kk in range(2):
            a_op[tc * 2 + kk, :, 2 * tc:2 * tc + 2] = W_opT[kk * 128:(kk + 1) * 128]
    a_op = np.ascontiguousarray(a_op.transpose(1, 0, 2))      # [128, 16, 16]

    stor = _np_stor()
    A = {
        "a_hid": ktiles(W_hid.T).astype(stor),                # [128, 2, 256]
        "a_gx0": ktiles(W_eff.T).astype(stor),                # [128, 2, 1024]
        "a_hh0": ktiles(W_hh0.T).astype(stor),                # [128, 2, 1024]
        "a_l1": ktiles(np.concatenate([W_ih1, W_hh1], 1).T).astype(stor),  # [128,4,1024]
        "a_ec": ktiles(np.concatenate([W_ep, W_conf], 0).T).astype(stor),  # [128,2,3]
        "a_op": a_op.astype(stor),                            # [128, 16, 16]
        "b_eff": np.ascontiguousarray(b_eff.reshape(8, 128).T).astype(np.float32),
        "b_g1": np.ascontiguousarray((b_ih1 + b_hh1).reshape(8, 128).T).astype(np.float32),
        "b_hid": np.ascontiguousarray(b_hid.reshape(2, 128).T).astype(np.float32),
        "b_ec": np.concatenate([b_ep, b_conf]).reshape(3, 1).astype(np.float32),
        "b_op": np.tile(b_op, GS).reshape(16, 1).astype(np.float32),
    }
    return A


def _build():
    """Build the per-core Bass program (identical on all cores)."""
    # dtype for matmul-feeding tensors: FP32r tensors must be *written* as
    # f32r (the verifier enforces producer-side rounding), so declare the
    # tiles and dram params with that dtype directly.
    stor = F32R if PRECISION == "f32" else BF16
    cdt = F32 if PRECISION == "f32" else BF16   # cell/activation tile dtype
    # gx0 feeds the PE identity-matmul (needs f32r) or only DVE adds (bf16 ok)
    gxdt = F32R if GX0_PE_PAIRS > 0 else (F32 if PRECISION == "f32" else BF16)

    nc = bacc.Bacc("TRN2", target_bir_lowering=False, debug=False)

    d_obs = nc.dram_tensor("obs", [BS_C, OBS], F32, kind="ExternalInput")
    d_lane = nc.dram_tensor("lane", [BS_C, LANE], F32, kind="ExternalInput")
    d_ds = nc.dram_tensor("ds", [K, DSH], F32, kind="ExternalInput")
    d_w = {}
    wshapes = {
        "a_hid": [128, 2, H], "a_gx0": [128, 2, H4], "a_hh0": [128, 2, H4],
        "a_l1": [128, 4, H4], "a_ec": [128, 2, 3], "a_op": [128, 16, 16],
    }
    for k, sh in wshapes.items():
        d_w[k] = nc.dram_tensor(k, sh, stor, kind="ExternalInput")
    bshapes = {"b_eff": [128, 8], "b_g1": [128, 8], "b_hid": [128, 2],
               "b_ec": [3, 1], "b_op": [16, 1]}
    for k, sh in bshapes.items():
        d_w[k] = nc.dram_tensor(k, sh, F32, kind="ExternalInput")

    d_traj = nc.dram_tensor("out_traj", [NGROUPS, 16, N], F32, kind="ExternalOutput")
    d_ec = nc.dram_tensor("out_ec", [3, N], F32, kind="ExternalOutput")

    with ExitStack() as ctx:
        tc_ = ctx.enter_context(tile.TileContext(nc))
        wpool = ctx.enter_context(tc_.tile_pool(name="weights", bufs=1))
        enc = ctx.enter_context(tc_.tile_pool(name="enc", bufs=1))
        gx0p = ctx.enter_context(tc_.tile_pool(name="gx0", bufs=1))
        hp0 = ctx.enter_context(tc_.tile_pool(name="h0", bufs=2))
        hp1 = ctx.enter_context(tc_.tile_pool(name="h1", bufs=2))
        cp = ctx.enter_context(tc_.tile_pool(name="cstate", bufs=1))
        actp = ctx.enter_context(tc_.tile_pool(name="acts", bufs=10))
        prep = ctx.enter_context(tc_.tile_pool(name="preact", bufs=3))
        tmpp = ctx.enter_context(tc_.tile_pool(name="tmp", bufs=2))
        tcp = ctx.enter_context(tc_.tile_pool(name="tanhc", bufs=2))
        stag = ctx.enter_context(tc_.tile_pool(name="stage", bufs=2))
        # PSUM: gate pool 3 x [128,768] tiles (2 banks each) + proj (2 banks)
        gp = ctx.enter_context(tc_.tile_pool(name="gpsum", bufs=3, space="PSUM"))
        pp = ctx.enter_context(tc_.tile_pool(name="ppsum", bufs=1, space="PSUM"))

        # ---- inputs first (tiny), then weights ordered by first use --------
        obs_sb = enc.tile([128, OBS], F32, tag="obs")
        nc.sync.dma_start(out=obs_sb, in_=d_obs[:])
        lane_sb = enc.tile([128, LANE], F32, tag="lane")
        nc.sync.dma_start(out=lane_sb, in_=d_lane[:])
        dsT = enc.tile([128, K], F32, tag="dsT")          # rows 64..127 used
        nc.sync.dma_start(out=dsT[64:128, :], in_=d_ds[:].rearrange("k d -> d k"))

        ident = wpool.tile([128, 128], F32, tag="ident")
        make_identity(nc, ident)
        identr = wpool.tile([128, 128], F32R, tag="identr")
        nc.vector.tensor_copy(identr, ident)

        w = {}
        for k, sh in bshapes.items():
            w[k] = wpool.tile(sh, F32, tag=k, name=k)
            nc.sync.dma_start(out=w[k], in_=d_w[k][:])
        order = ["a_hid", "a_ec", "a_gx0", "a_hh0", "a_l1", "a_op"]
        for k in order:
            w[k] = wpool.tile(wshapes[k], stor, tag=k, name=k)
            nc.sync.dma_start(out=w[k], in_=d_w[k][:])

        # ---- encoder -------------------------------------------------------

        # transposes via PE
        obsT_ps = gp.tile([128, 128], F32, tag="g")
        nc.tensor.transpose(obsT_ps, obs_sb, ident)
        obsT = enc.tile([128, 128], stor, tag="obsT")
        nc.vector.tensor_copy(obsT, obsT_ps)
        laneT_ps = gp.tile([64, 128], F32, tag="g")
        nc.tensor.transpose(laneT_ps, lane_sb, ident)
        laneT = enc.tile([64, 128], stor, tag="laneT")
        nc.vector.tensor_copy(laneT, laneT_ps)

        # combined^T [256, 768]: rows 0:128 obs, 128:192 lane, 192:256 ds
        combT = enc.tile([128, 2, N], stor, tag="combT")
        for k in range(K):
            blk = slice(k * 128, (k + 1) * 128)
            nc.vector.tensor_copy(combT[:, 0, blk], obsT)
            nc.vector.tensor_copy(combT[0:64, 1, blk], laneT)
            # broadcast ds[k] along the 128 batch columns (in0 * 0 + ds)
            nc.vector.tensor_scalar(
                out=combT[64:128, 1, blk], in0=obsT[64:128, 0:128],
                scalar1=0.0, scalar2=dsT[64:128, k:k + 1],
                op0=mybir.AluOpType.mult, op1=mybir.AluOpType.add)

        # hidden^T = LeakyRelu(W_hid @ combined^T + b_hid)   [256, 768]
        hidT = enc.tile([128, 2, N], stor, tag="hidT")
        for m in range(2):
            ps = gp.tile([128, N], F32, tag="g")
            for (n0, nw) in NCHUNKS:
                for kk in range(2):
                    nc.tensor.matmul(
                        ps[:, n0:n0 + nw], w["a_hid"][:, kk, m * 128:(m + 1) * 128],
                        combT[:, kk, n0:n0 + nw], start=(kk == 0), stop=(kk == 1))
            # leaky_relu(r, 0.1) = max(r, 0.1*r); Lrelu is unimplemented in sim
            r_ = enc.tile([128, N], F32, tag="enc_r", name="enc_r")
            nc.scalar.activation(r_, ps, IDENT, bias=w["b_hid"][:, m:m + 1])
            s_ = enc.tile([128, N], F32, tag="enc_s", name="enc_s")
            nc.vector.tensor_scalar_mul(s_, r_, 0.1)
            nc.vector.tensor_max(hidT[:, m, :], r_, s_)

        # endpoint / conf : [3, 768]
        ecps = gp.tile([3, N], F32, tag="g")
        for (n0, nw) in NCHUNKS:
            for kk in range(2):
                nc.tensor.matmul(ecps[:, n0:n0 + nw], w["a_ec"][:, kk, :],
                                 hidT[:, kk, n0:n0 + nw],
                                 start=(kk == 0), stop=(kk == 1))
        ec_st = stag.tile([3, N], F32, tag="ec")
        nc.scalar.activation(ec_st, ecps, IDENT, bias=w["b_ec"][:, 0:1])
        nc.sync.dma_start(out=d_ec[:], in_=ec_st)

        # gx0 = W_eff @ hidden^T + b_eff   [1024, 768] f32, kept in SBUF
        gx0 = gx0p.tile([128, 8, N], gxdt, tag="gx0")
        for m in range(8):
            ps = gp.tile([128, N], F32, tag="g")
            for (n0, nw) in NCHUNKS:
                for kk in range(2):
                    nc.tensor.matmul(ps[:, n0:n0 + nw],
                                     w["a_gx0"][:, kk, m * 128:(m + 1) * 128],
                                     hidT[:, kk, n0:n0 + nw],
                                     start=(kk == 0), stop=(kk == 1))
            nc.scalar.activation(gx0[:, m, :], ps, IDENT, bias=w["b_eff"][:, m:m + 1])

        # ---- LSTM loop -----------------------------------------------------
        cA = cp.tile([128, 2, N], cdt, tag="cA")   # layer0 cell state
        cB = cp.tile([128, 2, N], cdt, tag="cB")   # layer1 cell state
        proj_ps = [None]

        def cell(acts, c, h_pool, h_tag, t):
            """acts[m] m=0..7 (i,i,f,f,g,g,o,o); returns new h tile."""
            h_new = h_pool.tile([128, 2, N], stor, tag=h_tag)
            for kk in range(2):
                s_i, s_f, g_g = acts[0 + kk], acts[2 + kk], acts[4 + kk]
                if t == 0:
                    nc.vector.tensor_mul(c[:, kk, :], s_i, g_g)
                else:
                    tmp = tmpp.tile([128, N], cdt, tag="tmp")
                    nc.vector.tensor_mul(tmp, s_i, g_g)
                    nc.vector.tensor_mul(c[:, kk, :], c[:, kk, :], s_f)
                    nc.vector.tensor_add(c[:, kk, :], c[:, kk, :], tmp)
            tch = tcp.tile([128, 2, N], cdt, tag="tc")
            nc.scalar.activation(tch, c, TANH)     # fused [128, 1536]
            for kk in range(2):
                nc.vector.tensor_mul(h_new[:, kk, :], acts[6 + kk], tch[:, kk, :])
            return h_new

        def l0_step(t, h_prev):
            acts = []
            for m in range(8):
                func = TANH if m // 2 == 2 else SIG
                a = actp.tile([128, N], cdt, tag="act")
                if t == 0:
                    nc.scalar.activation(a, gx0[:, m, :], func)
                else:
                    ps = gp.tile([128, N], F32, tag="g")
                    use_pe = m < GX0_PE_PAIRS * 2
                    for (n0, nw) in NCHUNKS:
                        if use_pe:  # inject gx0 via identity matmul
                            nc.tensor.matmul(ps[:, n0:n0 + nw], identr,
                                             gx0[:, m, n0:n0 + nw],
                                             start=True, stop=False,
                                             skip_group_check=True)
                        for kk in range(2):
                            nc.tensor.matmul(
                                ps[:, n0:n0 + nw],
                                w["a_hh0"][:, kk, m * 128:(m + 1) * 128],
                                h_prev[:, kk, n0:n0 + nw],
                                start=(not use_pe and kk == 0), stop=(kk == 1),
                                skip_group_check=True)
                    if use_pe:
                        nc.scalar.activation(a, ps, func)
                    else:
                        pre = prep.tile([128, N], cdt, tag="pre")
                        nc.vector.tensor_add(pre, ps, gx0[:, m, :])
                        nc.scalar.activation(a, pre, func)
                acts.append(a)
            return cell(acts, cA, hp0, "h0", t)

        def l1_step(s, h_in, h1s):
            acts = []
            for m in range(8):
                func = TANH if m // 2 == 2 else SIG
                ps = gp.tile([128, N], F32, tag="g")
                nk = 2 if s == 0 else 4
                for (n0, nw) in NCHUNKS:
                    for kt in range(nk):
                        rhs = h_in if kt < 2 else h1s
                        kk = kt % 2
                        nc.tensor.matmul(ps[:, n0:n0 + nw],
                                         w["a_l1"][:, kt, m * 128:(m + 1) * 128],
                                         rhs[:, kk, n0:n0 + nw],
                                         start=(kt == 0), stop=(kt == nk - 1),
                                         skip_group_check=True)
                a = actp.tile([128, N], cdt, tag="act")
                nc.scalar.activation(a, ps, func, bias=w["b_g1"][:, m:m + 1])
                acts.append(a)
            h1_new = cell(acts, cB, hp1, "h1", s)

            # ---- time-blocked output projection ----
            g_, tcg = divmod(s, GS)
            gsz = min(GS, T - g_ * GS)
            if tcg == 0:
                proj_ps[0] = pp.tile([16, N], F32, tag="proj", name="proj")
            pps = proj_ps[0]
            for (n0, nw) in NCHUNKS:
                for kk in range(2):
                    nc.tensor.matmul(pps[:, n0:n0 + nw], w["a_op"][:, tcg * 2 + kk, :],
                                     h1_new[:, kk, n0:n0 + nw],
                                     start=(tcg == 0 and kk == 0),
                                     stop=(tcg == gsz - 1 and kk == 1),
                                     skip_group_check=True)
            if tcg == gsz - 1:
                st = stag.tile([16, N], F32, tag="traj")
                nc.scalar.activation(st, pps, IDENT, bias=w["b_op"][:, 0:1])
                nc.sync.dma_start(out=d_traj[g_], in_=st)
            return h1_new

        h0_prev = None   # h_l0(t-1)
        h1_prev = None   # h_l1(s-1)
        for t in range(T):
            h0_new = l0_step(t, h0_prev)
            if t >= 1:
                h1_prev = l1_step(t - 1, h0_prev, h1_prev)
            h0_prev = h0_new
        h1_prev = l1_step(T - 1, h0_prev, h1_prev)

    nc.compile()
    return nc


def _shard_inputs(inp, weights):
    obs = np.ascontiguousarray(np.asarray(inp["obs_feature"], np.float32))
    lane = np.ascontiguousarray(np.asarray(inp["lane_change_feature"], np.float32))
    ds = np.ascontiguousarray(np.asarray(inp["driving_style"], np.float32).reshape(K, DSH))
    in_maps = []
    for c in range(NCORES):
        m = dict(weights)
        m["obs"] = obs[c * BS_C:(c + 1) * BS_C]
        m["lane"] = lane[c * BS_C:(c + 1) * BS_C]
        m["ds"] = ds
        in_maps.append(m)
    return in_maps


def _assemble(results):
    final = np.empty((BS, K, T, 2), np.float32)
    conf = np.empty((BS, K), np.float32)
    endpoint = np.empty((BS, K, 2), np.float32)
    for c, r in enumerate(results):
        traj = np.asarray(r["out_traj"])          # [4, 16, 768]
        # [g, tc, f, k, bs] -> [bs, k, g, tc, f]
        arr = traj.reshape(NGROUPS, GS, 2, K, BS_C).transpose(4, 3, 0, 1, 2)
        final[c * BS_C:(c + 1) * BS_C] = arr.reshape(BS_C, K, NGROUPS * GS, 2)[:, :, :T]
        ec = np.asarray(r["out_ec"]).reshape(3, K, BS_C)     # [3, k, bs]
        endpoint[c * BS_C:(c + 1) * BS_C] = ec[:2].transpose(2, 1, 0)
        conf[c * BS_C:(c + 1) * BS_C] = ec[2].T
    return final, conf, endpoint


_CACHE = {}


def run(inputs, trace=False):
    """Returns ((final, conf, endpoint), BassKernelResults)."""
    assert int(inputs.get("out_length", T)) == T
    if "nc" not in _CACHE:
        _CACHE["nc"] = _build()
    nc = _CACHE["nc"]
    weights = _prep_host(inputs)
    in_maps = _shard_inputs(inputs, weights)
    res = run_bass_kernel_spmd(nc, in_maps, core_ids=list(range(NCORES)),
                               trace=trace)
    return _assemble(res.results), res


def kernel(**inputs):
    out, _ = run(inputs)
    return out


# revision 12
# speedup vs baseline: 1.2012x; 1.0027x over previous
"""AnchorBasedTrajectoryDecoder on 8 TRN2 NeuronCores (Bass/Tile).

Data-parallel: batch axis (1024) split into 8 shards of 128 rows; each core
runs all K=6 anchors for its rows => N = 768 independent LSTM rollouts per
core, laid out feature-major (columns j = k*128 + bs, "k-major").

Device program per core:
  encoder:  combined^T -> hidden^T (LeakyRelu) -> endpoint/conf, gx0
  loop t:   layer0 gates = gx0 + W_hh0 @ h0   (gx0 injected into PSUM by an
            identity matmul, recurrent matmuls accumulate on top)
            layer1 gates = [W_ih1|W_hh1] @ [h_l0(t); h_l1(t-1)]  (K=512)
            cell updates on ACT (sigmoid/tanh) + DVE (elementwise)
            output projection time-blocked 8 steps per PSUM accumulation
            via a block-diagonal W_op stack (M=16)
  layer1 is software-staggered one step behind layer0 so the ACT/DVE tail of
  each layer hides under the other layer's matmul block.
"""

import numpy as np
import ml_dtypes
from contextlib import ExitStack

import concourse.bacc as bacc
import concourse.bass as bass
import concourse.mybir as mybir
import concourse.tile as tile
from concourse.bass_utils import run_bass_kernel_spmd
from concourse.masks import make_identity

# problem constants (hardcoded from the spec)
BS, OBS, LANE, DSH = 1024, 128, 64, 64
IN_DIM, H, K, T = 256, 256, 6, 30
H4 = 4 * H
NCORES = 8
BS_C = BS // NCORES          # 128 batch rows per core
N = BS_C * K                 # 768 rollouts per core
NCHUNKS = ((0, 512), (512, 256))   # PSUM-bank-aligned column chunks of N
GS = 8                       # time-group size for the output projection
NGROUPS = (T + GS - 1) // GS

F32 = mybir.dt.float32
F32R = mybir.dt.float32r
BF16 = mybir.dt.bfloat16
AF = mybir.ActivationFunctionType
SIG, TANH, IDENT, LRELU = AF.Sigmoid, AF.Tanh, AF.Identity, AF.Lrelu

# --- config flags -----------------------------------------------------------
import os
# "f32": f32 storage + f32r matmuls, f32 cell (max precision)
# "bf16": bf16 matmuls (FWL weight loads) + bf16 cell (2x DVE modes)
PRECISION = os.environ.get("KPREC", "bf16")
# how many of the 8 layer-0 m-tiles inject gx0 via the PE identity-matmul
# (the rest use a DVE add from PSUM); bf16 mode defaults to all-DVE since
# the bf16 cell frees DVE capacity and the PE queue is the bottleneck
GX0_PE_PAIRS = int(os.environ.get("KGX0PE", "4" if PRECISION == "f32" else "0"))


def _np_stor():
    return np.float32 if PRECISION == "f32" else ml_dtypes.bfloat16


def _prep_host(inp):
    """Host-side weight re-layout (f64 math, cast to storage dtype)."""
    g = lambda k: np.asarray(inp[k], np.float64)
    W_hid, b_hid = g("W_hid"), g("b_hid")
    W_ep, b_ep = g("W_ep"), g("b_ep")
    W_conf, b_conf = g("W_conf"), g("b_conf")
    W_ih0, b_ih0 = g("W_ih0"), g("b_ih0")
    W_hh0, b_hh0 = g("W_hh0"), g("b_hh0")
    W_ih1, b_ih1 = g("W_ih1"), g("b_ih1")
    W_hh1, b_hh1 = g("W_hh1"), g("b_hh1")
    W_op, b_op = g("W_op"), g("b_op")

    # endpoint is a linear function of hidden; fold it into the layer-0 input
    # projection:  gx0 = W_eff @ hidden + b_eff
    W_eff = W_ih0[:, :H] + W_ih0[:, H:H + 2] @ W_ep          # [4H, H]
    b_eff = b_ih0 + b_hh0 + W_ih0[:, H:H + 2] @ b_ep         # [4H]

    def ktiles(WT):  # [K, M] -> [128, K/128, M]
        Kd, M = WT.shape
        assert Kd % 128 == 0
        return np.ascontiguousarray(WT.reshape(Kd // 128, 128, M).transpose(1, 0, 2))

    W_opT = W_op.T                                            # [H, 2]
    a_op = np.zeros((16, 128, 16))
    for tc in range(GS):
        for kk in range(2):
            a_op[tc * 2 + kk, :, 2 * tc:2 * tc + 2] = W_opT[kk * 128:(kk + 1) * 128]
    a_op = np.ascontiguousarray(a_op.transpose(1, 0, 2))      # [128, 16, 16]

    stor = _np_stor()
    A = {
        "a_hid": ktiles(W_hid.T).astype(stor),                # [128, 2, 256]
        "a_gx0": ktiles(W_eff.T).astype(stor),                # [128, 2, 1024]
        "a_hh0": ktiles(W_hh0.T).astype(stor),                # [128, 2, 1024]
        "a_l1": ktiles(np.concatenate([W_ih1, W_hh1], 1).T).astype(stor),  # [128,4,1024]
        "a_ec": ktiles(np.concatenate([W_ep, W_conf], 0).T).astype(stor),  # [128,2,3]
        "a_op": a_op.astype(stor),                            # [128, 16, 16]
        "b_eff": np.ascontiguousarray(b_eff.reshape(8, 128).T).astype(np.float32),
        "b_g1": np.ascontiguousarray((b_ih1 + b_hh1).reshape(8, 128).T).astype(np.float32),
        "b_hid": np.ascontiguousarray(b_hid.reshape(2, 128).T).astype(np.float32),
        "b_ec": np.concatenate([b_ep, b_conf]).reshape(3, 1).astype(np.float32),
        "b_op": np.tile(b_op, GS).reshape(16, 1).astype(np.float32),
    }
    return A


def _build():
    """Build the per-core Bass program (identical on all cores)."""
    # dtype for matmul-feeding tensors: FP32r tensors must be *written* as
    # f32r (the verifier enforces producer-side rounding), so declare the
    # tiles and dram params with that dtype directly.
    stor = F32R if PRECISION == "f32" else BF16
    cdt = F32 if PRECISION == "f32" else BF16   # cell/activation tile dtype
    # gx0 feeds the PE identity-matmul (needs f32r) or only DVE adds (bf16 ok)
    gxdt = F32R if GX0_PE_PAIRS > 0 else (F32 if PRECISION == "f32" else BF16)

    nc = bacc.Bacc("TRN2", target_bir_lowering=False, debug=False)

    d_obs = nc.dram_tensor("obs", [BS_C, OBS], F32, kind="ExternalInput")
    d_lane = nc.dram_tensor("lane", [BS_C, LANE], F32, kind="ExternalInput")
    d_ds = nc.dram_tensor("ds", [K, DSH], F32, kind="ExternalInput")
    d_w = {}
    wshapes = {
        "a_hid": [128, 2, H], "a_gx0": [128, 2, H4], "a_hh0": [128, 2, H4],
        "a_l1": [128, 4, H4], "a_ec": [128, 2, 3], "a_op": [128, 16, 16],
    }
    for k, sh in wshapes.items():
        d_w[k] = nc.dram_tensor(k, sh, stor, kind="ExternalInput")
    bshapes = {"b_eff": [128, 8], "b_g1": [128, 8], "b_hid": [128, 2],
               "b_ec": [3, 1], "b_op": [16, 1]}
    for k, sh in bshapes.items():
        d_w[k] = nc.dram_tensor(k, sh, F32, kind="ExternalInput")

    d_traj = nc.dram_tensor("out_traj", [NGROUPS, 16, N], F32, kind="ExternalOutput")
    d_ec = nc.dram_tensor("out_ec", [3, N], F32, kind="ExternalOutput")

    with ExitStack() as ctx:
        tc_ = ctx.enter_context(tile.TileContext(nc))
        wpool = ctx.enter_context(tc_.tile_pool(name="weights", bufs=1))
        enc = ctx.enter_context(tc_.tile_pool(name="enc", bufs=1))
        gx0p = ctx.enter_context(tc_.tile_pool(name="gx0", bufs=1))
        hp0 = ctx.enter_context(tc_.tile_pool(name="h0", bufs=2))
        hp1 = ctx.enter_context(tc_.tile_pool(name="h1", bufs=2))
        cp = ctx.enter_context(tc_.tile_pool(name="cstate", bufs=1))
        actp = ctx.enter_context(tc_.tile_pool(name="acts", bufs=10))
        prep = ctx.enter_context(tc_.tile_pool(name="preact", bufs=3))
        tmpp = ctx.enter_context(tc_.tile_pool(name="tmp", bufs=2))
        tcp = ctx.enter_context(tc_.tile_pool(name="tanhc", bufs=2))
        stag = ctx.enter_context(tc_.tile_pool(name="stage", bufs=2))
        # PSUM: gate pool 3 x [128,768] tiles (2 banks each) + proj (2 banks)
        gp = ctx.enter_context(tc_.tile_pool(name="gpsum", bufs=3, space="PSUM"))
        pp = ctx.enter_context(tc_.tile_pool(name="ppsum", bufs=1, space="PSUM"))

        # ---- inputs first (tiny), then weights ordered by first use --------
        obs_sb = enc.tile([128, OBS], F32, tag="obs")
        nc.sync.dma_start(out=obs_sb, in_=d_obs[:])
        lane_sb = enc.tile([128, LANE], F32, tag="lane")
        nc.sync.dma_start(out=lane_sb, in_=d_lane[:])
        dsT = enc.tile([128, K], F32, tag="dsT")          # rows 64..127 used
        nc.sync.dma_start(out=dsT[64:128, :], in_=d_ds[:].rearrange("k d -> d k"))

        ident = wpool.tile([128, 128], F32, tag="ident")
        make_identity(nc, ident)
        identr = wpool.tile([128, 128], F32R, tag="identr")
        nc.vector.tensor_copy(identr, ident)

        w = {}
        for k, sh in bshapes.items():
            w[k] = wpool.tile(sh, F32, tag=k, name=k)
            nc.sync.dma_start(out=w[k], in_=d_w[k][:])
        for k in ("a_hid", "a_ec", "a_gx0"):
            w[k] = wpool.tile(wshapes[k], stor, tag=k, name=k)
            nc.sync.dma_start(out=w[k], in_=d_w[k][:])
        for k in ("a_hh0", "a_l1", "a_op"):
            w[k] = wpool.tile(wshapes[k], stor, tag=k, name=k)
            nc.gpsimd.dma_start(out=w[k], in_=d_w[k][:])

        # ---- encoder -------------------------------------------------------

        # transposes via PE
        obsT_ps = gp.tile([128, 128], F32, tag="g")
        nc.tensor.transpose(obsT_ps, obs_sb, ident)
        obsT = enc.tile([128, 128], stor, tag="obsT")
        nc.vector.tensor_copy(obsT, obsT_ps)
        laneT_ps = gp.tile([64, 128], F32, tag="g")
        nc.tensor.transpose(laneT_ps, lane_sb, ident)
        laneT = enc.tile([64, 128], stor, tag="laneT")
        nc.vector.tensor_copy(laneT, laneT_ps)

        # combined^T [256, 768]: rows 0:128 obs, 128:192 lane, 192:256 ds
        combT = enc.tile([128, 2, N], stor, tag="combT")
        for k in range(K):
            blk = slice(k * 128, (k + 1) * 128)
            nc.vector.tensor_copy(combT[:, 0, blk], obsT)
            nc.vector.tensor_copy(combT[0:64, 1, blk], laneT)
            # broadcast ds[k] along the 128 batch columns (in0 * 0 + ds)
            nc.vector.tensor_scalar(
                out=combT[64:128, 1, blk], in0=obsT[64:128, 0:128],
                scalar1=0.0, scalar2=dsT[64:128, k:k + 1],
                op0=mybir.AluOpType.mult, op1=mybir.AluOpType.add)

        # hidden^T = LeakyRelu(W_hid @ combined^T + b_hid)   [256, 768]
        hidT = enc.tile([128, 2, N], stor, tag="hidT")
        for m in range(2):
            ps = gp.tile([128, N], F32, tag="g")
            for (n0, nw) in NCHUNKS:
                for kk in range(2):
                    nc.tensor.matmul(
                        ps[:, n0:n0 + nw], w["a_hid"][:, kk, m * 128:(m + 1) * 128],
                        combT[:, kk, n0:n0 + nw], start=(kk == 0), stop=(kk == 1))
            # leaky_relu(r, 0.1) = max(r, 0.1*r); Lrelu is unimplemented in sim
            r_ = enc.tile([128, N], F32, tag="enc_r", name="enc_r")
            nc.scalar.activation(r_, ps, IDENT, bias=w["b_hid"][:, m:m + 1])
            s_ = enc.tile([128, N], F32, tag="enc_s", name="enc_s")
            nc.vector.tensor_scalar_mul(s_, r_, 0.1)
            nc.vector.tensor_max(hidT[:, m, :], r_, s_)

        # endpoint / conf : [3, 768]
        ecps = gp.tile([3, N], F32, tag="g")
        for (n0, nw) in NCHUNKS:
            for kk in range(2):
                nc.tensor.matmul(ecps[:, n0:n0 + nw], w["a_ec"][:, kk, :],
                                 hidT[:, kk, n0:n0 + nw],
                                 start=(kk == 0), stop=(kk == 1))
        ec_st = stag.tile([3, N], F32, tag="ec")
        nc.scalar.activation(ec_st, ecps, IDENT, bias=w["b_ec"][:, 0:1])
        nc.sync.dma_start(out=d_ec[:], in_=ec_st)

        # gx0 = W_eff @ hidden^T + b_eff   [1024, 768], kept in SBUF.
        # The t=0 layer-0 activations are computed here straight from the
        # PSUM (ACT, with bias) while the DVE does the gx0 evacuation, so
        # the t=0 step costs no extra ACT serial time.
        gx0 = gx0p.tile([128, 8, N], gxdt, tag="gx0")
        t0_acts = []
        for m in range(8):
            ps = gp.tile([128, N], F32, tag="g")
            for (n0, nw) in NCHUNKS:
                for kk in range(2):
                    nc.tensor.matmul(ps[:, n0:n0 + nw],
                                     w["a_gx0"][:, kk, m * 128:(m + 1) * 128],
                                     hidT[:, kk, n0:n0 + nw],
                                     start=(kk == 0), stop=(kk == 1))
            a0 = actp.tile([128, N], cdt, tag="act", name="t0act")
            nc.scalar.activation(a0, ps, TANH if m // 2 == 2 else SIG,
                                 bias=w["b_eff"][:, m:m + 1])
            t0_acts.append(a0)
            nc.vector.tensor_scalar_add(gx0[:, m, :], ps, w["b_eff"][:, m:m + 1])

        # ---- LSTM loop -----------------------------------------------------
        cA = cp.tile([128, 2, N], cdt, tag="cA")   # layer0 cell state
        cB = cp.tile([128, 2, N], cdt, tag="cB")   # layer1 cell state
        proj_ps = [None]

        def cell(acts, c, h_pool, h_tag, t):
            """acts[m] m=0..7 (i,i,f,f,g,g,o,o); returns new h tile."""
            h_new = h_pool.tile([128, 2, N], stor, tag=h_tag)
            for kk in range(2):
                s_i, s_f, g_g = acts[0 + kk], acts[2 + kk], acts[4 + kk]
                if t == 0:
                    nc.vector.tensor_mul(c[:, kk, :], s_i, g_g)
                else:
                    tmp = tmpp.tile([128, N], cdt, tag="tmp")
                    nc.vector.tensor_mul(tmp, s_i, g_g)
                    nc.vector.tensor_mul(c[:, kk, :], c[:, kk, :], s_f)
                    nc.vector.tensor_add(c[:, kk, :], c[:, kk, :], tmp)
            tch = tcp.tile([128, 2, N], cdt, tag="tc")
            nc.scalar.activation(tch, c, TANH)     # fused [128, 1536]
            for kk in range(2):
                nc.vector.tensor_mul(h_new[:, kk, :], acts[6 + kk], tch[:, kk, :])
            return h_new

        def l0_step(t, h_prev):
            acts = []
            if t == 0:
                return cell(t0_acts, cA, hp0, "h0", t)
            for m in range(8):
                func = TANH if m // 2 == 2 else SIG
                a = actp.tile([128, N], cdt, tag="act")
                if True:
                    ps = gp.tile([128, N], F32, tag="g")
                    use_pe = m < GX0_PE_PAIRS * 2
                    for (n0, nw) in NCHUNKS:
                        if use_pe:  # inject gx0 via identity matmul
                            nc.tensor.matmul(ps[:, n0:n0 + nw], identr,
                                             gx0[:, m, n0:n0 + nw],
                                             start=True, stop=False,
                                             skip_group_check=True)
                        for kk in range(2):
                            nc.tensor.matmul(
                                ps[:, n0:n0 + nw],
                                w["a_hh0"][:, kk, m * 128:(m + 1) * 128],
                                h_prev[:, kk, n0:n0 + nw],
                                start=(not use_pe and kk == 0), stop=(kk == 1),
                                skip_group_check=True)
                    if use_pe:
                        nc.scalar.activation(a, ps, func)
                    else:
                        pre = prep.tile([128, N], cdt, tag="pre")
                        nc.vector.tensor_add(pre, ps, gx0[:, m, :])
                        nc.scalar.activation(a, pre, func)
                acts.append(a)
            return cell(acts, cA, hp0, "h0", t)

        def l1_step(s, h_in, h1s):
            acts = []
            for m in range(8):
                func = TANH if m // 2 == 2 else SIG
                ps = gp.tile([128, N], F32, tag="g")
                nk = 2 if s == 0 else 4
                for (n0, nw) in NCHUNKS:
                    for kt in range(nk):
                        rhs = h_in if kt < 2 else h1s
                        kk = kt % 2
                        nc.tensor.matmul(ps[:, n0:n0 + nw],
                                         w["a_l1"][:, kt, m * 128:(m + 1) * 128],
                                         rhs[:, kk, n0:n0 + nw],
                                         start=(kt == 0), stop=(kt == nk - 1),
                                         skip_group_check=True)
                a = actp.tile([128, N], cdt, tag="act")
                nc.scalar.activation(a, ps, func, bias=w["b_g1"][:, m:m + 1])
                acts.append(a)
            h1_new = cell(acts, cB, hp1, "h1", s)

            # ---- time-blocked output projection ----
            g_, tcg = divmod(s, GS)
            gsz = min(GS, T - g_ * GS)
            if tcg == 0:
                proj_ps[0] = pp.tile([16, N], F32, tag="proj", name="proj")
            pps = proj_ps[0]
            for (n0, nw) in NCHUNKS:
                for kk in range(2):
                    nc.tensor.matmul(pps[:, n0:n0 + nw], w["a_op"][:, tcg * 2 + kk, :],
                                     h1_new[:, kk, n0:n0 + nw],
                                     start=(tcg == 0 and kk == 0),
                                     stop=(tcg == gsz - 1 and kk == 1),
                                     skip_group_check=True)
            if tcg == gsz - 1:
                st = stag.tile([16, N], F32, tag="traj")
                nc.scalar.activation(st, pps, IDENT, bias=w["b_op"][:, 0:1])
                nc.sync.dma_start(out=d_traj[g_], in_=st)
            return h1_new

        h0_prev = None   # h_l0(t-1)
        h1_prev = None   # h_l1(s-1)
        for t in range(T):
            h0_new = l0_step(t, h0_prev)
            if t >= 1:
                h1_prev = l1_step(t - 1, h0_prev, h1_prev)
            h0_prev = h0_new
        h1_prev = l1_step(T - 1, h0_prev, h1_prev)

    nc.compile()
    return nc


def _shard_inputs(inp, weights):
    obs = np.ascontiguousarray(np.asarray(inp["obs_feature"], np.float32))
    lane = np.ascontiguousarray(np.asarray(inp["lane_change_feature"], np.float32))
    ds = np.ascontiguousarray(np.asarray(inp["driving_style"], np.float32).reshape(K, DSH))
    in_maps = []
    for c in range(NCORES):
        m = dict(weights)
        m["obs"] = obs[c * BS_C:(c + 1) * BS_C]
        m["lane"] = lane[c * BS_C:(c + 1) * BS_C]
        m["ds"] = ds
        in_maps.append(m)
    return in_maps


def _assemble(results):
    final = np.empty((BS, K, T, 2), np.float32)
    conf = np.empty((BS, K), np.float32)
    endpoint = np.empty((BS, K, 2), np.float32)
    for c, r in enumerate(results):
        traj = np.asarray(r["out_traj"])          # [4, 16, 768]
        # [g, tc, f, k, bs] -> [bs, k, g, tc, f]
        arr = traj.reshape(NGROUPS, GS, 2, K, BS_C).transpose(4, 3, 0, 1, 2)
        final[c * BS_C:(c + 1) * BS_C] = arr.reshape(BS_C, K, NGROUPS * GS, 2)[:, :, :T]
        ec = np.asarray(r["out_ec"]).reshape(3, K, BS_C)     # [3, k, bs]
        endpoint[c * BS_C:(c + 1) * BS_C] = ec[:2].transpose(2, 1, 0)
        conf[c * BS_C:(c + 1) * BS_C] = ec[2].T
    return final, conf, endpoint


_CACHE = {}


def run(inputs, trace=False):
    """Returns ((final, conf, endpoint), BassKernelResults)."""
    assert int(inputs.get("out_length", T)) == T
    if "nc" not in _CACHE:
        _CACHE["nc"] = _build()
    nc = _CACHE["nc"]
    weights = _prep_host(inputs)
    in_maps = _shard_inputs(inputs, weights)
    res = run_bass_kernel_spmd(nc, in_maps, core_ids=list(range(NCORES)),
                               trace=trace)
    return _assemble(res.results), res


def kernel(**inputs):
    out, _ = run(inputs)
    return out


# revision 13
# speedup vs baseline: 1.2083x; 1.0059x over previous
"""AnchorBasedTrajectoryDecoder on 8 TRN2 NeuronCores (Bass/Tile).

Data-parallel: batch axis (1024) split into 8 shards of 128 rows; each core
runs all K=6 anchors for its rows => N = 768 independent LSTM rollouts per
core, laid out feature-major (columns j = k*128 + bs, "k-major").

Device program per core:
  encoder:  combined^T -> hidden^T (LeakyRelu) -> endpoint/conf, gx0
  loop t:   layer0 gates = gx0 + W_hh0 @ h0   (gx0 injected into PSUM by an
            identity matmul, recurrent matmuls accumulate on top)
            layer1 gates = [W_ih1|W_hh1] @ [h_l0(t); h_l1(t-1)]  (K=512)
            cell updates on ACT (sigmoid/tanh) + DVE (elementwise)
            output projection time-blocked 8 steps per PSUM accumulation
            via a block-diagonal W_op stack (M=16)
  layer1 is software-staggered one step behind layer0 so the ACT/DVE tail of
  each layer hides under the other layer's matmul block.
"""

import numpy as np
import ml_dtypes
from contextlib import ExitStack

import concourse.bacc as bacc
import concourse.bass as bass
import concourse.mybir as mybir
import concourse.tile as tile
from concourse.bass_utils import run_bass_kernel_spmd
from concourse.masks import make_identity

# problem constants (hardcoded from the spec)
BS, OBS, LANE, DSH = 1024, 128, 64, 64
IN_DIM, H, K, T = 256, 256, 6, 30
H4 = 4 * H
NCORES = 8
BS_C = BS // NCORES          # 128 batch rows per core
N = BS_C * K                 # 768 rollouts per core
NCHUNKS = ((0, 512), (512, 256))   # PSUM-bank-aligned column chunks of N
GS = 8                       # time-group size for the output projection
NGROUPS = (T + GS - 1) // GS

F32 = mybir.dt.float32
F32R = mybir.dt.float32r
BF16 = mybir.dt.bfloat16
AF = mybir.ActivationFunctionType
SIG, TANH, IDENT, LRELU = AF.Sigmoid, AF.Tanh, AF.Identity, AF.Lrelu

# --- config flags -----------------------------------------------------------
import os
# "f32": f32 storage + f32r matmuls, f32 cell (max precision)
# "bf16": bf16 matmuls (FWL weight loads) + bf16 cell (2x DVE modes)
PRECISION = os.environ.get("KPREC", "bf16")
# how many of the 8 layer-0 m-tiles inject gx0 via the PE identity-matmul
# (the rest use a DVE add from PSUM); bf16 mode defaults to all-DVE since
# the bf16 cell frees DVE capacity and the PE queue is the bottleneck
GX0_PE_PAIRS = int(os.environ.get("KGX0PE", "4" if PRECISION == "f32" else "0"))


def _np_stor():
    return np.float32 if PRECISION == "f32" else ml_dtypes.bfloat16


def _prep_host(inp):
    """Host-side weight re-layout (f64 math, cast to storage dtype)."""
    g = lambda k: np.asarray(inp[k], np.float64)
    W_hid, b_hid = g("W_hid"), g("b_hid")
    W_ep, b_ep = g("W_ep"), g("b_ep")
    W_conf, b_conf = g("W_conf"), g("b_conf")
    W_ih0, b_ih0 = g("W_ih0"), g("b_ih0")
    W_hh0, b_hh0 = g("W_hh0"), g("b_hh0")
    W_ih1, b_ih1 = g("W_ih1"), g("b_ih1")
    W_hh1, b_hh1 = g("W_hh1"), g("b_hh1")
    W_op, b_op = g("W_op"), g("b_op")

    # endpoint is a linear function of hidden; fold it into the layer-0 input
    # projection:  gx0 = W_eff @ hidden + b_eff
    W_eff = W_ih0[:, :H] + W_ih0[:, H:H + 2] @ W_ep          # [4H, H]
    b_eff = b_ih0 + b_hh0 + W_ih0[:, H:H + 2] @ b_ep         # [4H]

    def ktiles(WT):  # [K, M] -> [128, K/128, M]
        Kd, M = WT.shape
        assert Kd % 128 == 0
        return np.ascontiguousarray(WT.reshape(Kd // 128, 128, M).transpose(1, 0, 2))

    W_opT = W_op.T                                            # [H, 2]
    a_op = np.zeros((16, 128, 16))
    for tc in range(GS):
        for kk in range(2):
            a_op[tc * 2 + kk, :, 2 * tc:2 * tc + 2] = W_opT[kk * 128:(kk + 1) * 128]
    a_op = np.ascontiguousarray(a_op.transpose(1, 0, 2))      # [128, 16, 16]

    stor = _np_stor()
    A = {
        "a_hid": ktiles(W_hid.T).astype(stor),                # [128, 2, 256]
        "a_gx0": ktiles(W_eff.T).astype(stor),                # [128, 2, 1024]
        "a_hh0": ktiles(W_hh0.T).astype(stor),                # [128, 2, 1024]
        "a_l1": ktiles(np.concatenate([W_ih1, W_hh1], 1).T).astype(stor),  # [128,4,1024]
        "a_ec": ktiles(np.concatenate([W_ep, W_conf], 0).T).astype(stor),  # [128,2,3]
        "a_op": a_op.astype(stor),                            # [128, 16, 16]
        "b_eff": np.ascontiguousarray(b_eff.reshape(8, 128).T).astype(np.float32),
        "b_g1": np.ascontiguousarray((b_ih1 + b_hh1).reshape(8, 128).T).astype(np.float32),
        "b_hid": np.ascontiguousarray(b_hid.reshape(2, 128).T).astype(np.float32),
        "b_ec": np.concatenate([b_ep, b_conf]).reshape(3, 1).astype(np.float32),
        "b_op": np.tile(b_op, GS).reshape(16, 1).astype(np.float32),
    }
    return A


def _build():
    """Build the per-core Bass program (identical on all cores)."""
    # dtype for matmul-feeding tensors: FP32r tensors must be *written* as
    # f32r (the verifier enforces producer-side rounding), so declare the
    # tiles and dram params with that dtype directly.
    stor = F32R if PRECISION == "f32" else BF16
    cdt = F32 if PRECISION == "f32" else BF16   # cell/activation tile dtype
    # gx0 feeds the PE identity-matmul (needs f32r) or only DVE adds (bf16 ok)
    gxdt = F32R if GX0_PE_PAIRS > 0 else (F32 if PRECISION == "f32" else BF16)

    nc = bacc.Bacc("TRN2", target_bir_lowering=False, debug=False)

    d_obs = nc.dram_tensor("obs", [BS_C, OBS], F32, kind="ExternalInput")
    d_lane = nc.dram_tensor("lane", [BS_C, LANE], F32, kind="ExternalInput")
    d_ds = nc.dram_tensor("ds", [K, DSH], F32, kind="ExternalInput")
    d_w = {}
    wshapes = {
        "a_hid": [128, 2, H], "a_gx0": [128, 2, H4], "a_hh0": [128, 2, H4],
        "a_l1": [128, 4, H4], "a_ec": [128, 2, 3], "a_op": [128, 16, 16],
    }
    for k, sh in wshapes.items():
        d_w[k] = nc.dram_tensor(k, sh, stor, kind="ExternalInput")
    bshapes = {"b_eff": [128, 8], "b_g1": [128, 8], "b_hid": [128, 2],
               "b_ec": [3, 1], "b_op": [16, 1]}
    for k, sh in bshapes.items():
        d_w[k] = nc.dram_tensor(k, sh, F32, kind="ExternalInput")

    d_traj = nc.dram_tensor("out_traj", [NGROUPS, 16, N], F32, kind="ExternalOutput")
    d_ec = nc.dram_tensor("out_ec", [3, N], F32, kind="ExternalOutput")

    with ExitStack() as ctx:
        tc_ = ctx.enter_context(tile.TileContext(nc))
        wpool = ctx.enter_context(tc_.tile_pool(name="weights", bufs=1))
        enc = ctx.enter_context(tc_.tile_pool(name="enc", bufs=1))
        gx0p = ctx.enter_context(tc_.tile_pool(name="gx0", bufs=1))
        hp0 = ctx.enter_context(tc_.tile_pool(name="h0", bufs=2))
        hp1 = ctx.enter_context(tc_.tile_pool(name="h1", bufs=2))
        cp = ctx.enter_context(tc_.tile_pool(name="cstate", bufs=1))
        actp = ctx.enter_context(tc_.tile_pool(name="acts", bufs=10))
        prep = ctx.enter_context(tc_.tile_pool(name="preact", bufs=3))
        tmpp = ctx.enter_context(tc_.tile_pool(name="tmp", bufs=2))
        tcp = ctx.enter_context(tc_.tile_pool(name="tanhc", bufs=2))
        stag = ctx.enter_context(tc_.tile_pool(name="stage", bufs=2))
        # PSUM: gate pool 3 x [128,768] tiles (2 banks each) + proj (2 banks)
        gp = ctx.enter_context(tc_.tile_pool(name="gpsum", bufs=3, space="PSUM"))
        pp = ctx.enter_context(tc_.tile_pool(name="ppsum", bufs=1, space="PSUM"))

        # ---- inputs first (tiny), then weights ordered by first use --------
        obs_sb = enc.tile([128, OBS], F32, tag="obs")
        nc.sync.dma_start(out=obs_sb, in_=d_obs[:])
        lane_sb = enc.tile([128, LANE], F32, tag="lane")
        nc.sync.dma_start(out=lane_sb, in_=d_lane[:])
        dsT = enc.tile([128, K], F32, tag="dsT")          # rows 64..127 used
        nc.sync.dma_start(out=dsT[64:128, :], in_=d_ds[:].rearrange("k d -> d k"))

        ident = wpool.tile([128, 128], F32, tag="ident")
        make_identity(nc, ident)
        identr = wpool.tile([128, 128], F32R, tag="identr")
        nc.vector.tensor_copy(identr, ident)

        w = {}
        for k in ("a_hid", "a_gx0", "a_ec"):
            w[k] = wpool.tile(wshapes[k], stor, tag=k, name=k)
            nc.sync.dma_start(out=w[k], in_=d_w[k][:])
        for k, sh in bshapes.items():
            w[k] = wpool.tile(sh, F32, tag=k, name=k)
            nc.gpsimd.dma_start(out=w[k], in_=d_w[k][:])
        for k in ("a_hh0", "a_l1", "a_op"):
            w[k] = wpool.tile(wshapes[k], stor, tag=k, name=k)
            nc.gpsimd.dma_start(out=w[k], in_=d_w[k][:])

        # ---- encoder -------------------------------------------------------

        # transposes via PE
        obsT_ps = gp.tile([128, 128], F32, tag="g")
        nc.tensor.transpose(obsT_ps, obs_sb, ident)
        obsT = enc.tile([128, 128], stor, tag="obsT")
        nc.vector.tensor_copy(obsT, obsT_ps)
        laneT_ps = gp.tile([64, 128], F32, tag="g")
        nc.tensor.transpose(laneT_ps, lane_sb, ident)
        laneT = enc.tile([64, 128], stor, tag="laneT")
        nc.vector.tensor_copy(laneT, laneT_ps)

        # combined^T [256, 768]: rows 0:128 obs, 128:192 lane, 192:256 ds
        combT = enc.tile([128, 2, N], stor, tag="combT")
        for k in range(K):
            blk = slice(k * 128, (k + 1) * 128)
            nc.vector.tensor_copy(combT[:, 0, blk], obsT)
            nc.vector.tensor_copy(combT[0:64, 1, blk], laneT)
            # broadcast ds[k] along the 128 batch columns (in0 * 0 + ds)
            nc.vector.tensor_scalar(
                out=combT[64:128, 1, blk], in0=obsT[64:128, 0:128],
                scalar1=0.0, scalar2=dsT[64:128, k:k + 1],
                op0=mybir.AluOpType.mult, op1=mybir.AluOpType.add)

        # hidden^T = LeakyRelu(W_hid @ combined^T + b_hid)   [256, 768]
        hidT = enc.tile([128, 2, N], stor, tag="hidT")
        for m in range(2):
            ps = gp.tile([128, N], F32, tag="g")
            for (n0, nw) in NCHUNKS:
                for kk in range(2):
                    nc.tensor.matmul(
                        ps[:, n0:n0 + nw], w["a_hid"][:, kk, m * 128:(m + 1) * 128],
                        combT[:, kk, n0:n0 + nw], start=(kk == 0), stop=(kk == 1))
            # leaky_relu(r, 0.1) = max(r, 0.1*r); Lrelu is unimplemented in sim
            r_ = enc.tile([128, N], F32, tag="enc_r", name="enc_r")
            nc.scalar.activation(r_, ps, IDENT, bias=w["b_hid"][:, m:m + 1])
            s_ = enc.tile([128, N], F32, tag="enc_s", name="enc_s")
            nc.vector.tensor_scalar_mul(s_, r_, 0.1)
            nc.vector.tensor_max(hidT[:, m, :], r_, s_)

        # endpoint / conf : [3, 768]
        ecps = gp.tile([3, N], F32, tag="g")
        for (n0, nw) in NCHUNKS:
            for kk in range(2):
                nc.tensor.matmul(ecps[:, n0:n0 + nw], w["a_ec"][:, kk, :],
                                 hidT[:, kk, n0:n0 + nw],
                                 start=(kk == 0), stop=(kk == 1))
        ec_st = stag.tile([3, N], F32, tag="ec")
        nc.scalar.activation(ec_st, ecps, IDENT, bias=w["b_ec"][:, 0:1])
        nc.sync.dma_start(out=d_ec[:], in_=ec_st)

        # gx0 = W_eff @ hidden^T + b_eff   [1024, 768], kept in SBUF.
        # The t=0 layer-0 activations are computed here straight from the
        # PSUM (ACT, with bias) while the DVE does the gx0 evacuation, so
        # the t=0 step costs no extra ACT serial time.
        gx0 = gx0p.tile([128, 8, N], gxdt, tag="gx0")
        t0_acts = []
        for m in range(8):
            ps = gp.tile([128, N], F32, tag="g")
            for (n0, nw) in NCHUNKS:
                for kk in range(2):
                    nc.tensor.matmul(ps[:, n0:n0 + nw],
                                     w["a_gx0"][:, kk, m * 128:(m + 1) * 128],
                                     hidT[:, kk, n0:n0 + nw],
                                     start=(kk == 0), stop=(kk == 1))
            a0 = actp.tile([128, N], cdt, tag="act", name="t0act")
            nc.scalar.activation(a0, ps, TANH if m // 2 == 2 else SIG,
                                 bias=w["b_eff"][:, m:m + 1])
            t0_acts.append(a0)
            nc.vector.tensor_scalar_add(gx0[:, m, :], ps, w["b_eff"][:, m:m + 1])

        # ---- LSTM loop -----------------------------------------------------
        cA = cp.tile([128, 2, N], cdt, tag="cA")   # layer0 cell state
        cB = cp.tile([128, 2, N], cdt, tag="cB")   # layer1 cell state
        proj_ps = [None]

        def cell(acts, c, h_pool, h_tag, t):
            """acts[m] m=0..7 (i,i,f,f,g,g,o,o); returns new h tile."""
            h_new = h_pool.tile([128, 2, N], stor, tag=h_tag)
            for kk in range(2):
                s_i, s_f, g_g = acts[0 + kk], acts[2 + kk], acts[4 + kk]
                if t == 0:
                    nc.vector.tensor_mul(c[:, kk, :], s_i, g_g)
                else:
                    tmp = tmpp.tile([128, N], cdt, tag="tmp")
                    nc.vector.tensor_mul(tmp, s_i, g_g)
                    nc.vector.tensor_mul(c[:, kk, :], c[:, kk, :], s_f)
                    nc.vector.tensor_add(c[:, kk, :], c[:, kk, :], tmp)
            tch = tcp.tile([128, 2, N], cdt, tag="tc")
            nc.scalar.activation(tch, c, TANH)     # fused [128, 1536]
            for kk in range(2):
                nc.vector.tensor_mul(h_new[:, kk, :], acts[6 + kk], tch[:, kk, :])
            return h_new

        def l0_step(t, h_prev):
            acts = []
            if t == 0:
                return cell(t0_acts, cA, hp0, "h0", t)
            # per gate pair: 2 psum fills + 2 DVE gx0-adds into a pair-shaped
            # SBUF pre tile, then ONE fused sigma/tanh ACT [128, 1536]
            pair_acts = []
            for p in range(4):
                func = TANH if p == 2 else SIG
                pre = prep.tile([128, 2, N], cdt, tag="pre")
                for sub in range(2):
                    m = 2 * p + sub
                    ps = gp.tile([128, N], F32, tag="g")
                    for (n0, nw) in NCHUNKS:
                        for kk in range(2):
                            nc.tensor.matmul(
                                ps[:, n0:n0 + nw],
                                w["a_hh0"][:, kk, m * 128:(m + 1) * 128],
                                h_prev[:, kk, n0:n0 + nw],
                                start=(kk == 0), stop=(kk == 1),
                                skip_group_check=True)
                    nc.vector.tensor_add(pre[:, sub, :], ps, gx0[:, m, :])
                a = actp.tile([128, 2, N], cdt, tag="actp", name="actp")
                nc.scalar.activation(a, pre, func)
                pair_acts.append(a)
            acts = []
            for p in range(4):
                acts.append(pair_acts[p][:, 0, :])
                acts.append(pair_acts[p][:, 1, :])
            acts = [acts[0], acts[1], acts[2], acts[3], acts[4], acts[5], acts[6], acts[7]]
            # reorder to (i,i,f,f,g,g,o,o) kk-interleaved: acts[m] with m= gate*2+kk
            return cell(acts, cA, hp0, "h0", t)

        def l1_step(s, h_in, h1s):
            acts = []
            for m in range(8):
                func = TANH if m // 2 == 2 else SIG
                ps = gp.tile([128, N], F32, tag="g")
                nk = 2 if s == 0 else 4
                for (n0, nw) in NCHUNKS:
                    for kt in range(nk):
                        rhs = h_in if kt < 2 else h1s
                        kk = kt % 2
                        nc.tensor.matmul(ps[:, n0:n0 + nw],
                                         w["a_l1"][:, kt, m * 128:(m + 1) * 128],
                                         rhs[:, kk, n0:n0 + nw],
                                         start=(kt == 0), stop=(kt == nk - 1),
                                         skip_group_check=True)
                a = actp.tile([128, N], cdt, tag="act")
                nc.scalar.activation(a, ps, func, bias=w["b_g1"][:, m:m + 1])
                acts.append(a)
            h1_new = cell(acts, cB, hp1, "h1", s)

            # ---- time-blocked output projection ----
            g_, tcg = divmod(s, GS)
            gsz = min(GS, T - g_ * GS)
            if tcg == 0:
                proj_ps[0] = pp.tile([16, N], F32, tag="proj", name="proj")
            pps = proj_ps[0]
            for (n0, nw) in NCHUNKS:
                for kk in range(2):
                    nc.tensor.matmul(pps[:, n0:n0 + nw], w["a_op"][:, tcg * 2 + kk, :],
                                     h1_new[:, kk, n0:n0 + nw],
                                     start=(tcg == 0 and kk == 0),
                                     stop=(tcg == gsz - 1 and kk == 1),
                                     skip_group_check=True)
            if tcg == gsz - 1:
                st = stag.tile([16, N], F32, tag="traj")
                nc.scalar.activation(st, pps, IDENT, bias=w["b_op"][:, 0:1])
                nc.sync.dma_start(out=d_traj[g_], in_=st)
            return h1_new

        h0_prev = None   # h_l0(t-1)
        h1_prev = None   # h_l1(s-1)
        for t in range(T):
            h0_new = l0_step(t, h0_prev)
            if t >= 1:
                h1_prev = l1_step(t - 1, h0_prev, h1_prev)
            h0_prev = h0_new
        h1_prev = l1_step(T - 1, h0_prev, h1_prev)

    nc.compile()
    return nc


def _shard_inputs(inp, weights):
    obs = np.ascontiguousarray(np.asarray(inp["obs_feature"], np.float32))
    lane = np.ascontiguousarray(np.asarray(inp["lane_change_feature"], np.float32))
    ds = np.ascontiguousarray(np.asarray(inp["driving_style"], np.float32).reshape(K, DSH))
    in_maps = []
    for c in range(NCORES):
        m = dict(weights)
        m["obs"] = obs[c * BS_C:(c + 1) * BS_C]
        m["lane"] = lane[c * BS_C:(c + 1) * BS_C]
        m["ds"] = ds
        in_maps.append(m)
    return in_maps


def _assemble(results):
    final = np.empty((BS, K, T, 2), np.float32)
    conf = np.empty((BS, K), np.float32)
    endpoint = np.empty((BS, K, 2), np.float32)
    for c, r in enumerate(results):
        traj = np.asarray(r["out_traj"])          # [4, 16, 768]
        # [g, tc, f, k, bs] -> [bs, k, g, tc, f]
        arr = traj.reshape(NGROUPS, GS, 2, K, BS_C).transpose(4, 3, 0, 1, 2)
        final[c * BS_C:(c + 1) * BS_C] = arr.reshape(BS_C, K, NGROUPS * GS, 2)[:, :, :T]
        ec = np.asarray(r["out_ec"]).reshape(3, K, BS_C)     # [3, k, bs]
        endpoint[c * BS_C:(c + 1) * BS_C] = ec[:2].transpose(2, 1, 0)
        conf[c * BS_C:(c + 1) * BS_C] = ec[2].T
    return final, conf, endpoint


_CACHE = {}


def run(inputs, trace=False):
    """Returns ((final, conf, endpoint), BassKernelResults)."""
    assert int(inputs.get("out_length", T)) == T
    if "nc" not in _CACHE:
        _CACHE["nc"] = _build()
    nc = _CACHE["nc"]
    weights = _prep_host(inputs)
    in_maps = _shard_inputs(inputs, weights)
    res = run_bass_kernel_spmd(nc, in_maps, core_ids=list(range(NCORES)),
                               trace=trace)
    return _assemble(res.results), res


def kernel(**inputs):
    out, _ = run(inputs)
    return out
